# revision 24
# baseline (speedup 1.0000x reference)
"""Nystrom attention (nn_Attention2) Trainium2 Bass kernel, v3.

Sharding: 8 cores = 4 batches x 2 head-groups (4 heads each).
Host combines: out[b] = partial[2b] + partial[2b+1] + x[b] + b_out.

v3 strategy (vs v2): fp8 DoubleRow for every big matmul, Act does only
exps (+some phase-A work), elementwise rebalanced across DVE/Pool.
  - LN: bn_stats on DVE; rstd = exp(-0.5*ln(var+eps)) on Act (one act
    table: Identity/Ln/Exp); apply split Act(Identity+bias)/Pool.
  - xhat stored fp8; transposed via uint16-view XBAR DMA -> d-pair layout
    with d = 256*pc + 2*p + g, enabling fp8 DoubleRow (K=256) for the
    qkv projections (host reorders W rows to match).
  - q/k scaled x16 (fp8-friendly); exps use scale=1/2048 (= /8/16/16).
  - scores (e1/e3/attn2) run DR with a zeros group: q/k tiles are
    [128, 2, NT] with block1 zeroed -> fixed group stride NT.
  - o2 folds the attn3 denominators in via a ones column (lhsT M=96).
  - conv as fp8-DR banded matmuls accumulated into the psO psum tile;
    one scalar_tensor_tensor per (chunk, head) assembles out_nat fp8.
  - to_out: out_nat fp8 -> u16 transpose -> one DR matmul per chunk.
PSUM budget (8 banks): pBig 2x4KB (ps3/psE/psO) + pA2 2x2KB (qkv) +
pO2 2x1KB (o2) + pMid 1x2KB (everything else) = 16KB.
"""

import sys

sys.path.insert(0, "/opt/trn_rl_repo")

import numpy as np

import concourse.bass as bass
import concourse.bacc as bacc
import concourse.tile as tile
from concourse import mybir
from concourse.bass_utils import run_bass_kernel_spmd

F32 = mybir.dt.float32
BF16 = mybir.dt.bfloat16
FP8 = mybir.dt.float8e4
U16 = mybir.dt.uint16
DR = mybir.MatmulPerfMode.DoubleRow
AF = mybir.ActivationFunctionType
OP = mybir.AluOpType

NT = 4096  # tokens
D = 512  # model dim
HC = 4  # heads per core
DH = 64  # head dim
M = 256  # landmarks
L = 16  # pool group
KW = 33  # conv kernel
EPS = 1e-5
SQ, SK, SV, SO = 16.0, 16.0, 8.0, 8.0
ES = 1.0 / (8.0 * SQ * SK)  # exp scale undoing wq/wk scaling + dh^-0.5


def build_kernel_body(tc):
    nc = tc.nc
    lp = nc.allow_low_precision(reason="fp8 DR pipeline; validated end-to-end")
    lp.__enter__()

    x = nc.dram_tensor("x", [NT, D], BF16, kind="ExternalInput").ap()
    wqk = nc.dram_tensor("wqk", [128, 2, 2, 512], FP8, kind="ExternalInput").ap()
    wv = nc.dram_tensor("wv", [128, 2, 2, 256], FP8, kind="ExternalInput").ap()
    wout = nc.dram_tensor("wout", [128, 2, 512], FP8, kind="ExternalInput").ap()
    bandsd = nc.dram_tensor("bands", [128, HC, 4, 128], FP8, kind="ExternalInput").ap()
    alphaI = nc.dram_tensor("alphaI", [3, 2, 128, 256], BF16, kind="ExternalInput").ap()
    ident = nc.dram_tensor("ident", [128, 128], BF16, kind="ExternalInput").ap()
    poolm = nc.dram_tensor("poolm", [128, 8], BF16, kind="ExternalInput").ap()
    zeros8 = nc.dram_tensor("zeros8", [128, NT], FP8, kind="ExternalInput").ap()
    out_p = nc.dram_tensor("out_partial", [NT, D], BF16, kind="ExternalOutput").ap()

    with tc.tile_pool(name="consts", bufs=1) as consts, tc.tile_pool(
        name="persist", bufs=1
    ) as persist:
        wqk_sb = consts.tile([128, 2, 2, 512], FP8, tag="wqk")
        wv_sb = consts.tile([128, 2, 2, 256], FP8, tag="wv")
        wout_sb = consts.tile([128, 2, 512], FP8, tag="wout")
        bands_sb = consts.tile([128, HC, 4, 128], FP8, tag="bands")
        aI_sb = consts.tile([128, 3, 2, 256], BF16, tag="aI")
        ident_sb = consts.tile([128, 128], BF16, tag="ident")
        poolm_sb = consts.tile([128, 8], BF16, tag="poolm")
        nc.sync.dma_start(out=poolm_sb, in_=poolm)
        nc.sync.dma_start(out=ident_sb, in_=ident)
        ones128 = consts.tile([128, 1], BF16, tag="ones128")
        nc.vector.memset(ones128, 1.0)
        ones_row = consts.tile([1, 128], BF16, tag="ones_row")
        nc.vector.memset(ones_row, 1.0)

        # persistent tensors
        xpT = persist.tile([128, 2, NT], U16, tag="xpT")  # xhat d-pairs, transposed
        qT = [persist.tile([128, 2, NT], FP8, tag=f"qT{i}", name=f"qT{i}") for i in range(2)]
        kT = [persist.tile([128, 2, NT], FP8, tag=f"kT{i}", name=f"kT{i}") for i in range(2)]
        v_nat = persist.tile([128, 35, HC, 96], FP8, tag="v_nat")
        xlp = persist.tile([128, 2, 2, M], FP8, tag="xlp")  # pooled xhat, d-pairs
        qlT = [persist.tile([128, 2, M], FP8, tag=f"qlT{i}", name=f"qlT{i}") for i in range(2)]
        klT = [persist.tile([128, 2, M], FP8, tag=f"klT{i}", name=f"klT{i}") for i in range(2)]
        out_nat = persist.tile([128, 32, HC, DH], FP8, tag="out_nat")
        outT = persist.tile([128, NT], U16, tag="outT")
        mvall = persist.tile([128, 2, 32], F32, tag="mvall")
        rstd = persist.tile([128, 32], F32, tag="rstd")
        nmur = persist.tile([128, 32], F32, tag="nmur")

        xpT8 = xpT.bitcast(FP8)  # [128, 2, 2*NT]: (p, pc, (t,g))
        outT8 = outT.bitcast(FP8)  # [128, 2*NT]: (p, (t,g))
        kTflat = [kT[i].rearrange("p c t -> p (c t)") for i in range(2)]

        # ---------------- Phase A1: LN -> xhat fp8 -> transpose + pooling ----
        with tc.tile_pool(name="xpool", bufs=32) as pX, tc.tile_pool(
            name="xhpool", bufs=12
        ) as pXH, tc.tile_pool(name="stpool", bufs=4) as pST, tc.tile_pool(
            name="ps_pool", bufs=2, space="PSUM"
        ) as psum_pool:
            xts = []
            for t in range(32):
                xt = pX.tile([128, D], BF16, tag="xt", name=f"xt{t}")
                nc.sync.dma_start(
                    out=xt, in_=x.rearrange("(c p) d -> p c d", p=128)[:, t, :]
                )
                xts.append(xt)
            # heavy consts + init DMAs after x loads (x is the critical path)
            nc.sync.dma_start(out=wqk_sb, in_=wqk)
            nc.sync.dma_start(out=wv_sb, in_=wv)
            nc.sync.dma_start(out=bands_sb, in_=bandsd)
            nc.sync.dma_start(out=aI_sb, in_=alphaI.rearrange("a c p j -> p a c j"))
            nc.sync.dma_start(out=wout_sb, in_=wout)
            for i in range(2):
                nc.sync.dma_start(out=qT[i][:, 1, :], in_=zeros8)
                nc.sync.dma_start(out=kT[i][:, 1, :], in_=zeros8)
                nc.gpsimd.memset(qlT[i][:, 1, :], 0.0)
                nc.gpsimd.memset(klT[i][:, 1, :], 0.0)
            nc.gpsimd.memset(v_nat[:, 0, :, :], 0.0)
            nc.gpsimd.memset(v_nat[:, 33:35, :, :], 0.0)
            nc.gpsimd.memset(v_nat[:, 1:33, :, 64:96], 0.0)
            nc.gpsimd.memset(v_nat[:, 1:33, :, 64:65], 1.0)
            for t in range(32):
                stats = pST.tile([128, 6], F32, tag="stats", name=f"st{t}")
                nc.vector.bn_stats(out=stats, in_=xts[t])
                nc.vector.bn_aggr(out=mvall[:, :, t], in_=stats)
            vpe = pST.tile([128, 32], F32, tag="vpe")
            nc.vector.tensor_scalar_add(vpe, mvall[:, 1, :], EPS)
            lnv = pST.tile([128, 32], F32, tag="lnv")
            nc.scalar.activation(lnv, vpe, AF.Ln)
            nc.scalar.activation(rstd, lnv, AF.Exp, scale=-0.5)
            nc.vector.scalar_tensor_tensor(
                nmur, mvall[:, 0, :], -1.0, rstd, OP.mult, OP.mult,
            )
            for blk in range(4):
                b8 = slice(blk * 8, blk * 8 + 8)
                psL = psum_pool.tile([1, 64], F32, tag="psL", name=f"psL{blk}")
                psL_sb = pST.tile([1, 64], BF16, tag="psLsb", name=f"psLsb{blk}")
                for tt in range(8):
                    t = blk * 8 + tt
                    nmurb = pST.tile([128, 1], BF16, tag="nmurb", name=f"nmurb{t}")
                    nc.gpsimd.tensor_copy(nmurb, nmur[:, t:t+1])
                    nc.tensor.matmul(
                        psL[:, tt * 8:(tt + 1) * 8], nmurb, poolm_sb,
                        start=True, stop=True,
                    )
                nc.vector.tensor_copy(psL_sb, psL)
                for tt in range(8):
                    t = blk * 8 + tt
                    pmr = pST.tile([128, 8], BF16, tag="pmr", name=f"pmr{t}")
                    nc.gpsimd.tensor_scalar_mul(pmr, poolm_sb, rstd[:, t:t+1])
                    psp = psum_pool.tile([128, 2, 2, 8], F32, tag="psp", name=f"psp{t}")
                    for pc in range(2):
                        for g in range(2):
                            sel = xts[t][:, pc * 256 : pc * 256 + 256].rearrange(
                                "p (m two) -> p m two", two=2
                            )[:, :, g]
                            nc.tensor.matmul(
                                psp[:, pc, g, :], sel, pmr,
                                start=True, stop=False,
                            )
                            nc.tensor.matmul(
                                psp[:, pc, g, :], ones_row,
                                psL_sb[:, tt * 8:(tt + 1) * 8],
                                start=False, stop=True,
                            )
                    nc.vector.tensor_copy(xlp[:, :, :, t * 8:(t + 1) * 8], psp)
                # LN applies + transposes for this blk (3-way engine split)
                for tt in range(8):
                    t = blk * 8 + tt
                    xh = pXH.tile([128, D], FP8, tag="xh", name=f"xh{t}")
                    if t % 8 < 2:
                        nc.scalar.activation(
                            xh, xts[t], AF.Identity,
                            bias=nmur[:, t:t+1], scale=rstd[:, t:t+1],
                        )
                    else:
                        nc.gpsimd.tensor_scalar(
                            xh, xts[t], mvall[:, 0, t:t+1], rstd[:, t:t+1],
                            OP.subtract, OP.mult,
                        )
                    nc.sync.dma_start_transpose(
                        out=xpT[:, :, t * 128:(t + 1) * 128], in_=xh.bitcast(U16)
                    )

        # ---------------- phase functions ------------------------------------
        HS = {}

        def lm_proj(pMid):
            # landmark q/k projections: psq rows = both heads of the pair
            for cc in range(4):
                dst = (qlT if cc < 2 else klT)[cc % 2]
                psq = pMid.tile([128, M], F32, tag="mid", name=f"psq{cc}")
                for pc in range(2):
                    nc.tensor.matmul(
                        psq,
                        wqk_sb[:, pc, :, (cc // 2) * 256 + (cc % 2) * 128 : (cc // 2) * 256 + (cc % 2) * 128 + 128],
                        xlp[:, pc, :, :],
                        start=(pc == 0), stop=(pc == 1), perf_mode=DR,
                    )
                nc.vector.tensor_copy(dst[:, 0, :], psq)

        def qkv_gen(pA2):
            rr = [0]

            def evac(out, in_, scale=None):
                rr[0] += 1
                if rr[0] % 2 == 0:
                    if scale is None:
                        nc.vector.tensor_copy(out, in_)
                    else:
                        nc.vector.tensor_scalar_mul(out, in_, scale)
                else:
                    nc.scalar.activation(
                        out, in_, AF.Copy, scale=1.0 if scale is None else scale
                    )

            for t8 in range(8):
                sp = t8 * 512
                for cc in range(4):
                    ps = pA2.tile([128, 512], F32, tag="a2", name=f"psqk{t8}_{cc}")
                    rhs = xpT8[:, :, 2 * sp : 2 * sp + 1024].rearrange(
                        "p c (t g) -> p c g t", g=2
                    )
                    for pc in range(2):
                        nc.tensor.matmul(
                            ps,
                            wqk_sb[:, pc, :, (cc // 2) * 256 + (cc % 2) * 128 : (cc // 2) * 256 + (cc % 2) * 128 + 128],
                            rhs[:, pc, :, :],
                            start=(pc == 0), stop=(pc == 1), perf_mode=DR,
                        )
                    dst = (qT if cc < 2 else kT)[cc % 2]
                    evac(dst[:, 0, sp:sp + 512], ps)
                for tv2 in range(2):
                    psv = pA2.tile([128, 2, 256], F32, tag="a2", name=f"psv{t8}_{tv2}")
                    for ti in range(2):
                        tv = t8 * 4 + tv2 * 2 + ti
                        for g in range(2):
                            lhsT = xpT8[
                                :, :, 2 * tv * 128 : 2 * tv * 128 + 256
                            ].rearrange("p c (t g) -> p c g t", g=2)[:, :, g, :]
                            nc.tensor.matmul(
                                psv[:, ti, :], lhsT, wv_sb[:, g, :, :],
                                start=(g == 0), stop=(g == 1), perf_mode=DR,
                            )
                    tv0 = t8 * 4 + tv2 * 2
                    evac(
                        v_nat[:, tv0 + 1: tv0 + 3, :, 0:64],
                        psv.rearrange("p c (h d) -> p c h d", h=HC),
                        scale=1.0 / SV,
                    )
                yield

        def ph_attn2(h, pMid, pS):
            st = HS[h]
            ps2 = pMid.tile([128, 2, 256], F32, tag="mid", name=f"ps2_{h}")
            for ic in range(2):
                nc.tensor.matmul(
                    ps2[:, ic, :],
                    st["qlTh"][:, :, ic * 128:(ic + 1) * 128],
                    st["klTh"],
                    start=True, stop=True, perf_mode=DR,
                )
            e2 = pS.tile([128, 2, 256], BF16, tag="e2", name=f"e2_{h}")
            nc.scalar.activation(e2, ps2, AF.Exp, scale=ES)
            rs = pS.tile([128, 2], F32, tag="rs", name=f"rs_{h}")
            nc.vector.tensor_reduce(rs, e2, mybir.AxisListType.X, OP.add)
            rr2 = pS.tile([128, 2], F32, tag="rr2", name=f"rr2_{h}")
            nc.vector.reciprocal(rr2, rs)
            a_nat = [
                st["pa"].tile([128, M], BF16, tag=f"anat{ic}", name=f"anat{h}_{ic}")
                for ic in range(2)
            ]
            rr2b = pS.tile([128, 2], F32, tag="rr2b", name=f"rr2b_{h}")
            nc.vector.tensor_copy(rr2b, rr2)
            for ic in range(2):
                nc.gpsimd.tensor_scalar_mul(a_nat[ic], e2[:, ic, :], rr2b[:, ic:ic+1])
            st["a_nat"] = a_nat

        def ph_z0(h, pMid, pS):
            st = HS[h]
            a_nat = st["a_nat"]
            pZ, pa = st["pZ"], st["pa"]
            psc = pMid.tile([1, M], F32, tag="mid", name=f"psc_{h}")
            for ic in range(2):
                nc.tensor.matmul(
                    psc, ones128, a_nat[ic], start=(ic == 0), stop=(ic == 1)
                )
            cmax = pS.tile([1, 1], F32, tag="cmax", name=f"cmax_{h}")
            nc.vector.tensor_reduce(cmax, psc, mybir.AxisListType.X, OP.max)
            crec = pS.tile([1, 1], BF16, tag="crec", name=f"crec_{h}")
            nc.vector.reciprocal(crec, cmax)
            crec_b = pS.tile([128, 1], F32, tag="crecb", name=f"crecb_{h}")
            psb_ = pMid.tile([128, 128], F32, tag="mid", name=f"psb_{h}")
            nc.tensor.matmul(psb_[:, 0:1], ones_row, crec, start=True, stop=True)
            nc.vector.tensor_copy(crec_b, psb_[:, 0:1])
            aT = pa.tile([128, 2, M], BF16, tag="aT", name=f"aT_{h}")
            z = pZ.tile([128, 2, M], BF16, tag="z", name=f"z_{h}")
            zT = pZ.tile([128, 2, M], BF16, tag="zT", name=f"zT_{h}")
            for jc in range(2):
                pT = pMid.tile([128, 2, 128], BF16, tag="mid", name=f"pTa_{h}_{jc}")
                for ic in range(2):
                    nc.tensor.transpose(
                        pT[:, ic, :], a_nat[ic][:, jc * 128:(jc + 1) * 128], ident_sb
                    )
                nc.vector.tensor_copy(aT[:, jc, :], pT.rearrange("p a b -> p (a b)"))
                nc.gpsimd.tensor_scalar_mul(z[:, jc, :], aT[:, jc, :], crec_b)
            for ic in range(2):
                nc.gpsimd.tensor_scalar_mul(zT[:, ic, :], a_nat[ic], crec_b)
            st["aT"], st["z"], st["zT"] = aT, z, zT

        def e3_scores(h, quad, pBig, pE3):
            st = HS[h]
            hp, ki = st["hp"], st["pair"]
            ps3 = pBig.tile([128, 4, 256], F32, tag="big", name=f"ps3_{h}_{quad}")
            for i in range(4):
                c = quad * 4 + i
                lhsT = kTflat[ki][hp:hp + 64, c * 128 : c * 128 + 256].rearrange(
                    "p (c2 t) -> p c2 t", c2=2
                )
                nc.tensor.matmul(
                    ps3[:, i, :], lhsT, st["qlTh"],
                    start=True, stop=True, perf_mode=DR,
                )
            e3q = pE3.tile([128, 4, 256], FP8, tag="e3q", name=f"e3q_{h}_{quad}")
            nc.scalar.activation(e3q, ps3, AF.Exp, scale=ES)
            st["e3qs"][quad] = e3q

        def e3_o2(h, quad):
            st = HS[h]
            e3q = st["e3qs"].pop(quad)
            o2ps = st["o2ps"]
            for i2 in range(2):
                pr = quad * 2 + i2
                nc.tensor.matmul(
                    o2ps,
                    v_nat[:, 2 * pr + 1 : 2 * pr + 3, h, :],
                    e3q[:, 2 * i2 : 2 * i2 + 2, :],
                    start=(pr == 0), stop=(pr == 15), perf_mode=DR,
                )

        def ph_e3_fin(h, pMid, pS):
            st = HS[h]
            o2ps = st["o2ps"]
            rrow = pS.tile([1, M], BF16, tag="rrow", name=f"rrow_{h}")
            nc.vector.reciprocal(rrow, o2ps[64:65, :])
            rs3 = [
                pS.tile([128, 1], F32, tag=f"rs3{jc}", name=f"rs3_{h}_{jc}")
                for jc in range(2)
            ]
            for jc in range(2):
                pT = pMid.tile([128, 128], BF16, tag="mid", name=f"pTf_{h}_{jc}")
                nc.tensor.transpose(
                    pT[:, 0:1], rrow[:, jc * 128:(jc + 1) * 128], ident_sb[0:1, 0:1]
                )
                nc.vector.tensor_scalar_mul(rs3[jc], pT[:, 0:1], 0.25)
            o2sb = pS.tile([64, M], BF16, tag="o2sb", name=f"o2sb_{h}")
            nc.vector.tensor_copy(o2sb, o2ps[0:64, :])
            o2n = [
                pS.tile([128, DH], BF16, tag=f"o2n{jc}", name=f"o2n_{h}_{jc}")
                for jc in range(2)
            ]
            for jc in range(2):
                pT = pMid.tile([128, 128], BF16, tag="mid", name=f"pTg_{h}_{jc}")
                nc.tensor.transpose(
                    pT[:, 0:64], o2sb[:, jc * 128:(jc + 1) * 128],
                    ident_sb[0:64, 0:64],
                )
                nc.vector.tensor_scalar_mul(o2n[jc], pT[:, 0:64], rs3[jc])
            st["o2n"] = o2n

        def ph_pinv_iter(h, pMid):
            st = HS[h]
            aT, z, zT = st["aT"], st["z"], st["zT"]
            pU = st["pU"]
            azT = pU.tile([128, 2, M], BF16, tag="u", name=f"azT_{h}")
            u1 = pU.tile([128, 2, M], BF16, tag="u", name=f"u1_{h}")
            ps_az = pMid.tile([128, 2, M], F32, tag="mid", name=f"ps_az_{h}")
            for oc in range(2):
                for kc in range(2):
                    nc.tensor.matmul(
                        ps_az[:, oc, :],
                        aT[:, kc, oc * 128:(oc + 1) * 128],
                        z[:, kc, :],
                        start=(kc == 0), stop=(kc == 1),
                    )
            nc.vector.tensor_tensor(
                u1.rearrange("p a b -> p (a b)"),
                aI_sb[:, 0, :, :].rearrange("p a b -> p (a b)"),
                ps_az.rearrange("p a b -> p (a b)"),
                OP.subtract,
            )
            ps_azT = pMid.tile([128, 2, M], F32, tag="mid", name=f"ps_azT_{h}")
            for oc in range(2):
                for kc in range(2):
                    nc.tensor.matmul(
                        ps_azT[:, oc, :],
                        z[:, kc, oc * 128:(oc + 1) * 128],
                        aT[:, kc, :],
                        start=(kc == 0), stop=(kc == 1),
                    )
            nc.vector.tensor_copy(
                azT.rearrange("p a b -> p (a b)"), ps_azT.rearrange("p a b -> p (a b)")
            )
            u2 = pU.tile([128, 2, M], BF16, tag="u", name=f"u2_{h}")
            ps_p1 = pMid.tile([128, 2, M], F32, tag="mid", name=f"ps_p1_{h}")
            for oc in range(2):
                for kc in range(2):
                    nc.tensor.matmul(
                        ps_p1[:, oc, :],
                        azT[:, kc, oc * 128:(oc + 1) * 128],
                        u1[:, kc, :],
                        start=(kc == 0), stop=(kc == 1),
                    )
            nc.vector.tensor_tensor(
                u2.rearrange("p a b -> p (a b)"),
                aI_sb[:, 1, :, :].rearrange("p a b -> p (a b)"),
                ps_p1.rearrange("p a b -> p (a b)"),
                OP.subtract,
            )
            u3 = pU.tile([128, 2, M], BF16, tag="u", name=f"u3_{h}")
            ps_p2 = pMid.tile([128, 2, M], F32, tag="mid", name=f"ps_p2_{h}")
            for oc in range(2):
                for kc in range(2):
                    nc.tensor.matmul(
                        ps_p2[:, oc, :],
                        azT[:, kc, oc * 128:(oc + 1) * 128],
                        u2[:, kc, :],
                        start=(kc == 0), stop=(kc == 1),
                    )
            nc.vector.tensor_tensor(
                u3.rearrange("p a b -> p (a b)"),
                aI_sb[:, 2, :, :].rearrange("p a b -> p (a b)"),
                ps_p2.rearrange("p a b -> p (a b)"),
                OP.subtract,
            )
            zTn = st["pZ"].tile([128, 2, M], BF16, tag="zT", name=f"zTn_{h}")
            ps_zTn = pMid.tile([128, 2, M], F32, tag="mid", name=f"ps_zTn_{h}")
            for oc in range(2):
                for kc in range(2):
                    nc.tensor.matmul(
                        ps_zTn[:, oc, :],
                        u3[:, kc, oc * 128:(oc + 1) * 128],
                        zT[:, kc, :],
                        start=(kc == 0), stop=(kc == 1),
                    )
            nc.vector.tensor_copy(
                zTn.rearrange("p a b -> p (a b)"),
                ps_zTn.rearrange("p a b -> p (a b)"),
            )
            st["zT"] = zTn

        def ph_C(h, pMid):
            st = HS[h]
            zT, o2n = st["zT"], st["o2n"]
            Cp = st["pa"].tile([128, 2, 96], FP8, tag="Cp", name=f"Cp_{h}")
            nc.gpsimd.memset(Cp[:, :, 64:96], 0.0)
            nc.gpsimd.memset(Cp[:, :, 64:65], 1.0)
            for ic in range(2):
                ps = pMid.tile([128, 64], F32, tag="mid", name=f"psC_{h}_{ic}")
                for jc in range(2):
                    nc.tensor.matmul(
                        ps,
                        zT[:, jc, ic * 128:(ic + 1) * 128],
                        o2n[jc],
                        start=(jc == 0), stop=(jc == 1),
                    )
                nc.vector.tensor_copy(Cp[:, ic, 0:64], ps)
            st["Cp"] = Cp

        def e1_t8(h, t8, pBig, pE1):
            st = HS[h]
            hp, ki = st["hp"], st["pair"]
            sp = t8 * 512
            psE = pBig.tile([128, 2, 512], F32, tag="big", name=f"psE_{h}_{t8}")
            for jc in range(2):
                nc.tensor.matmul(
                    psE[:, jc, :],
                    st["klTh"][:, :, jc * 128:(jc + 1) * 128],
                    qT[ki][hp:hp + 64, :, sp:sp + 512],
                    start=True, stop=True, perf_mode=DR,
                )
            e1q = pE1.tile([128, 2, 512], FP8, tag="e1q", name=f"e1q_{h}_{t8}")
            nc.scalar.activation(e1q, psE, AF.Exp, scale=ES)
            st["e1qs"][t8] = e1q

        def out_chunk(pair, heads, ck, pA2, pS):
            t8, tq = ck // 4, ck % 4
            psO = pA2.tile([128, 2, 160], F32, tag="a2", name=f"psO_{pair}_{ck}")
            for hi, h in enumerate(heads):
                st = HS[h]
                e1q = st["e1qs"][t8]
                nc.tensor.matmul(
                    psO[:, hi, 0:96],
                    e1q[:, :, tq * 128:(tq + 1) * 128],
                    st["Cp"],
                    start=True, stop=True, perf_mode=DR,
                )
                for p2 in range(2):
                    nc.tensor.matmul(
                        psO[:, hi, 96:160],
                        bands_sb[:, h, 2 * p2 : 2 * p2 + 2, :],
                        v_nat[:, ck + 2 * p2 : ck + 2 * p2 + 2, h, 0:64],
                        start=(p2 == 0), stop=(p2 == 1), perf_mode=DR,
                    )
            cnv = pS.tile([128, 2, DH], BF16, tag="cnv", name=f"cnv_{pair}_{ck}")
            if ck % 2 == 0:
                nc.vector.tensor_copy(cnv, psO[:, :, 96:160])
            else:
                nc.scalar.activation(cnv, psO[:, :, 96:160], AF.Copy)
            rr = pS.tile([128, 2], F32, tag="rrO", name=f"rrO_{pair}_{ck}")
            nc.vector.reciprocal(rr, psO[:, :, 64])
            for hi, h in enumerate(heads):
                nc.vector.scalar_tensor_tensor(
                    out_nat[:, ck, h, :],
                    psO[:, hi, 0:64],
                    rr[:, hi:hi + 1],
                    cnv[:, hi, :],
                    OP.mult,
                    OP.add,
                )

        def e1_pair(pair, heads, pBig, pE1, pA2, pS):
            for h in heads:
                HS[h]["e1qs"] = {}
            for h in heads:
                e1_t8(h, 0, pBig, pE1)
            for t8 in range(8):
                if t8 < 7:
                    for h in heads:
                        e1_t8(h, t8 + 1, pBig, pE1)
                for tq in range(4):
                    out_chunk(pair, heads, t8 * 4 + tq, pA2, pS)
                yield

        def fin_chain(heads, pMid, pS):
            for h in heads:
                ph_e3_fin(h, pMid, pS)
                yield
            for h in heads:
                ph_C(h, pMid)
                yield

        def e3_pair(heads, pBig, pE3, pO2):
            o2pair = pO2.tile([96, 2, 256], F32, tag="o2", name=f"o2ps_{heads[0]}")
            for h in heads:
                HS[h]["o2ps"] = o2pair[:, h % 2, :]
                HS[h]["e3qs"] = {}
            for h in heads:
                e3_scores(h, 0, pBig, pE3)
            for quad in range(8):
                for h in heads:
                    if quad < 7:
                        e3_scores(h, quad + 1, pBig, pE3)
                    e3_o2(h, quad)
                yield

        def drain_gen(pMid, pO2, pFO):
            opr = out_p.rearrange("(c p) d -> p c d", p=128)
            for t8 in range(8):
                ck0 = t8 * 4
                nc.sync.dma_start_transpose(
                    out=outT[:, ck0 * 128:(ck0 + 4) * 128].rearrange(
                        "p (c t) -> p c t", c=4
                    ),
                    in_=out_nat[:, ck0:ck0 + 4, :, :].rearrange(
                        "p c h d -> p (c h d)"
                    ).bitcast(U16),
                )
                for ck2 in range(2 * t8, 2 * t8 + 2):
                    fo = pFO.tile([128, 2, 512], BF16, tag="fo", name=f"fo{ck2}")
                    for ci in range(2):
                        ck = 2 * ck2 + ci
                        pool_ = pMid if ck % 2 == 0 else pO2
                        tag_ = "mid" if ck % 2 == 0 else "o2"
                        psF = pool_.tile([128, 512], F32, tag=tag_, name=f"psF_{ck}")
                        lhsT2 = outT8[:, 2 * ck * 128 : 2 * ck * 128 + 256].rearrange(
                            "p (t g) -> p g t", g=2
                        )
                        for g in range(2):
                            nc.tensor.matmul(
                                psF, lhsT2[:, g, :], wout_sb[:, g, :],
                                start=(g == 0), stop=(g == 1),
                            )
                        if ck % 2 == 0:
                            nc.vector.tensor_scalar_mul(fo[:, ci, :], psF, 1.0 / SO)
                        else:
                            nc.scalar.activation(
                                fo[:, ci, :], psF, AF.Copy, scale=1.0 / SO
                            )
                    nc.sync.dma_start(out=opr[:, 2 * ck2 : 2 * ck2 + 2, :], in_=fo)
                yield

        # ---------------- heads phase orchestration --------------------------
        with tc.tile_pool(name="ps_big", bufs=2, space="PSUM") as pBig, tc.tile_pool(
            name="ps_a2", bufs=2, space="PSUM"
        ) as pA2, tc.tile_pool(
            name="ps_o2", bufs=1, space="PSUM"
        ) as pO2, tc.tile_pool(
            name="ps_mid", bufs=1, space="PSUM"
        ) as pMid, tc.tile_pool(name="headS", bufs=8) as pS, tc.tile_pool(
            name="head_a", bufs=6
        ) as pa_pool, tc.tile_pool(name="pinv_u", bufs=6) as pU, tc.tile_pool(
            name="pinv_z", bufs=6
        ) as pZ, tc.tile_pool(name="e1pool", bufs=6) as pE1, tc.tile_pool(
            name="e3pool", bufs=4
        ) as pE3, tc.tile_pool(name="fo", bufs=4) as pFO:
            for h in range(4):
                pair, hp = h // 2, 64 * (h % 2)
                HS[h] = {
                    "pair": pair, "hp": hp,
                    "qlTh": qlT[pair][hp:hp + 64, :, :],
                    "klTh": klT[pair][hp:hp + 64, :, :],
                    "pZ": pZ, "pU": pU, "pa": pa_pool,
                }

            lm_proj(pMid)
            gqkv = qkv_gen(pA2)
            next(gqkv, None)
            next(gqkv, None)
            next(gqkv, None)
            for h in range(4):
                ph_attn2(h, pMid, pS)
            for h in range(4):
                ph_z0(h, pMid, pS)
            for h in range(4):
                ph_pinv_iter(h, pMid)
            # e3 pair0 || qkv
            g_e3a = e3_pair([0, 1], pBig, pE3, pO2)
            for quad in range(8):
                next(g_e3a, None)
                next(gqkv, None)
            # e3 pair1 || fin pair0 || qkv tail
            g_e3b = e3_pair([2, 3], pBig, pE3, pO2)
            g_fin0 = fin_chain([0, 1], pMid, pS)
            for quad in range(8):
                next(g_e3b, None)
                next(g_fin0, None)
                next(gqkv, None)
            for _ in g_fin0:
                pass
            for _ in gqkv:
                pass
            # e1 pair0 || fin pair1
            g_e1a = e1_pair(0, [0, 1], pBig, pE1, pA2, pS)
            g_fin1 = fin_chain([2, 3], pMid, pS)
            for t8 in range(8):
                next(g_e1a, None)
                next(g_fin1, None)
            for _ in g_fin1:
                pass
            # e1 pair1 || drain
            g_e1b = e1_pair(1, [2, 3], pBig, pE1, pA2, pS)
            g_dr = drain_gen(pMid, pO2, pFO)
            for t8 in range(8):
                next(g_e1b, None)
                next(g_dr, None)
            for _ in g_dr:
                pass
    lp.__exit__(None, None, None)


_NC_CACHE = None


def build_nc():
    global _NC_CACHE
    if _NC_CACHE is not None:
        return _NC_CACHE
    nc = bacc.Bacc("TRN2", target_bir_lowering=False, debug=False, num_devices=8)
    with tile.TileContext(nc) as tc:
        build_kernel_body(tc)
    nc.compile()
    _NC_CACHE = nc
    return nc


def host_inputs(x, w_qkv, w_out, b_out, res_w, ln_g, ln_b):
    """Build the 8 per-core input maps."""
    assert np.abs(ln_b).max() == 0.0, "nonzero ln_b not supported"
    import ml_dtypes

    bf16 = ml_dtypes.bfloat16
    fp8 = ml_dtypes.float8_e4m3
    eye = np.eye(M, dtype=np.float32)
    alphaI = np.stack(
        [a * eye.reshape(2, 128, M) for a in (7.0, 15.0, 13.0)]
    ).astype(bf16)
    identity = np.eye(128, dtype=bf16)
    poolm = np.zeros((128, 8), dtype=np.float32)
    for t in range(128):
        poolm[t, t // L] = 1.0 / L
    poolm = poolm.astype(bf16)
    zeros8 = np.zeros((128, NT), dtype=fp8)

    p_idx = np.arange(128)
    tp = np.arange(128)[:, None]
    t_ = np.arange(128)[None, :]
    in_maps = []
    for c in range(8):
        b, g = c // 2, c % 2
        qsl = slice(g * 256, g * 256 + 256)
        ksl = slice(512 + g * 256, 512 + g * 256 + 256)
        vsl = slice(1024 + g * 256, 1024 + g * 256 + 256)
        wq = (ln_g[:, None] * w_qkv[:, qsl]) * SQ  # [512, 256]
        wk = (ln_g[:, None] * w_qkv[:, ksl]) * SK
        wv_ = (ln_g[:, None] * w_qkv[:, vsl]) * SV
        # d-pair layout: [p, pc, gg, col], d = 256*pc + 2*p + gg
        wqk_c = np.zeros((128, 2, 2, 512), dtype=np.float32)
        wv_c = np.zeros((128, 2, 2, 256), dtype=np.float32)
        for pc in range(2):
            for gg in range(2):
                rows = 256 * pc + 2 * p_idx + gg
                wqk_c[:, pc, gg, 0:256] = wq[rows, :]
                wqk_c[:, pc, gg, 256:512] = wk[rows, :]
                wv_c[:, gg, pc, :] = wv_[rows, :]
        # wout: hd-pair layout [p, gg, :], hd = 2*p + gg (within group g)
        wout_c = np.zeros((128, 2, 512), dtype=np.float32)
        for gg in range(2):
            wout_c[:, gg, :] = w_out[g * 256 + 2 * p_idx + gg, :] * SO
        bands = np.zeros((128, HC, 4, 128), dtype=np.float32)
        for i in range(HC):
            w33 = res_w[4 * g + i, 0, :, 0]
            for pos, off in ((0, -128), (1, 0), (2, 128)):
                k = (tp + off) - t_ + 16
                msk = (k >= 0) & (k < KW)
                bands[:, i, pos][msk] = w33[k[msk]]
        in_maps.append(
            {
                "x": np.ascontiguousarray(x[b], dtype=bf16),
                "wqk": wqk_c.astype(fp8),
                "wv": wv_c.astype(fp8),
                "wout": wout_c.astype(fp8),
                "bands": bands.astype(fp8),
                "alphaI": alphaI,
                "ident": identity,
                "poolm": poolm,
                "zeros8": zeros8,
            }
        )
    return in_maps


def run(inputs, trace=False):
    nc = build_nc()
    in_maps = host_inputs(**inputs)
    res = run_bass_kernel_spmd(nc, in_maps, list(range(8)), trace=trace)
    x = inputs["x"]
    b_out = inputs["b_out"]
    out = np.stack(
        [
            res.results[2 * b]["out_partial"].astype(np.float32)
            + res.results[2 * b + 1]["out_partial"].astype(np.float32)
            for b in range(4)
        ]
    )
    out = out + x + b_out[None, None, :]
    return out.astype(np.float32), res


def kernel(**inputs):
    out, _ = run(inputs, trace=False)
    return out


# revision 26
# speedup vs baseline: 1.0795x; 1.0795x over previous
"""Nystrom attention (nn_Attention2) Trainium2 Bass kernel, v3.

Sharding: 8 cores = 4 batches x 2 head-groups (4 heads each).
Host combines: out[b] = partial[2b] + partial[2b+1] + x[b] + b_out.

v3 strategy (vs v2): fp8 DoubleRow for every big matmul, Act does only
exps (+some phase-A work), elementwise rebalanced across DVE/Pool.
  - LN: bn_stats on DVE; rstd = exp(-0.5*ln(var+eps)) on Act (one act
    table: Identity/Ln/Exp); apply split Act(Identity+bias)/Pool.
  - xhat stored fp8; transposed via uint16-view XBAR DMA -> d-pair layout
    with d = 256*pc + 2*p + g, enabling fp8 DoubleRow (K=256) for the
    qkv projections (host reorders W rows to match).
  - q/k scaled x16 (fp8-friendly); exps use scale=1/2048 (= /8/16/16).
  - scores (e1/e3/attn2) run DR with a zeros group: q/k tiles are
    [128, 2, NT] with block1 zeroed -> fixed group stride NT.
  - o2 folds the attn3 denominators in via a ones column (lhsT M=96).
  - conv as fp8-DR banded matmuls accumulated into the psO psum tile;
    one scalar_tensor_tensor per (chunk, head) assembles out_nat fp8.
  - to_out: out_nat fp8 -> u16 transpose -> one DR matmul per chunk.
PSUM budget (8 banks): pBig 2x4KB (ps3/psE/psO) + pA2 2x2KB (qkv) +
pO2 2x1KB (o2) + pMid 1x2KB (everything else) = 16KB.
"""

import sys

sys.path.insert(0, "/opt/trn_rl_repo")

import numpy as np

import concourse.bass as bass
import concourse.bacc as bacc
import concourse.tile as tile
from concourse import mybir
from concourse.bass_utils import run_bass_kernel_spmd

F32 = mybir.dt.float32
BF16 = mybir.dt.bfloat16
FP8 = mybir.dt.float8e4
U16 = mybir.dt.uint16
DR = mybir.MatmulPerfMode.DoubleRow
AF = mybir.ActivationFunctionType
OP = mybir.AluOpType

NT = 4096  # tokens
D = 512  # model dim
HC = 4  # heads per core
DH = 64  # head dim
M = 256  # landmarks
L = 16  # pool group
KW = 33  # conv kernel
EPS = 1e-5
SQ, SK, SV, SO = 16.0, 16.0, 8.0, 8.0
ES = 1.0 / (8.0 * SQ * SK)  # exp scale undoing wq/wk scaling + dh^-0.5


def build_kernel_body(tc):
    nc = tc.nc
    lp = nc.allow_low_precision(reason="fp8 DR pipeline; validated end-to-end")
    lp.__enter__()

    x = nc.dram_tensor("x", [NT, D], BF16, kind="ExternalInput").ap()
    wqk = nc.dram_tensor("wqk", [128, 2, 2, 512], FP8, kind="ExternalInput").ap()
    wv = nc.dram_tensor("wv", [128, 2, 2, 256], FP8, kind="ExternalInput").ap()
    wout = nc.dram_tensor("wout", [128, 2, 512], FP8, kind="ExternalInput").ap()
    bandsd = nc.dram_tensor("bands", [128, HC, 4, 128], FP8, kind="ExternalInput").ap()
    alphaI = nc.dram_tensor("alphaI", [3, 2, 128, 256], BF16, kind="ExternalInput").ap()
    ident = nc.dram_tensor("ident", [128, 128], BF16, kind="ExternalInput").ap()
    poolm = nc.dram_tensor("poolm", [128, 8], BF16, kind="ExternalInput").ap()
    zeros8 = nc.dram_tensor("zeros8", [128, NT], FP8, kind="ExternalInput").ap()
    out_p = nc.dram_tensor("out_partial", [NT, D], BF16, kind="ExternalOutput").ap()

    with tc.tile_pool(name="consts", bufs=1) as consts, tc.tile_pool(
        name="persist", bufs=1
    ) as persist:
        wqk_sb = consts.tile([128, 2, 2, 512], FP8, tag="wqk")
        wv_sb = consts.tile([128, 2, 2, 256], FP8, tag="wv")
        wout_sb = consts.tile([128, 2, 512], FP8, tag="wout")
        bands_sb = consts.tile([128, HC, 4, 128], FP8, tag="bands")
        aI_sb = consts.tile([128, 3, 2, 256], BF16, tag="aI")
        ident_sb = consts.tile([128, 128], BF16, tag="ident")
        poolm_sb = consts.tile([128, 8], BF16, tag="poolm")
        nc.sync.dma_start(out=poolm_sb, in_=poolm)
        nc.sync.dma_start(out=ident_sb, in_=ident)
        ones128 = consts.tile([128, 1], BF16, tag="ones128")
        nc.vector.memset(ones128, 1.0)
        ones_row = consts.tile([1, 128], BF16, tag="ones_row")
        nc.vector.memset(ones_row, 1.0)
        neg_row = consts.tile([1, 128], BF16, tag="neg_row")
        nc.vector.memset(neg_row, -1.0)

        # persistent tensors
        xpT = persist.tile([128, 2, NT], U16, tag="xpT")  # xhat d-pairs, transposed
        qT = [persist.tile([128, 2, NT], FP8, tag=f"qT{i}", name=f"qT{i}") for i in range(2)]
        kT = [persist.tile([128, 2, NT], FP8, tag=f"kT{i}", name=f"kT{i}") for i in range(2)]
        v_nat = persist.tile([128, 35, HC, 96], FP8, tag="v_nat")
        xlp = persist.tile([128, 2, 2, M], FP8, tag="xlp")  # pooled xhat, d-pairs
        qlT = [persist.tile([128, 2, M], FP8, tag=f"qlT{i}", name=f"qlT{i}") for i in range(2)]
        klT = [persist.tile([128, 2, M], FP8, tag=f"klT{i}", name=f"klT{i}") for i in range(2)]
        out_nat = persist.tile([128, 32, HC, DH], FP8, tag="out_nat")
        outT = persist.tile([128, NT], U16, tag="outT")
        mvall = persist.tile([128, 2, 32], F32, tag="mvall")
        rstd = persist.tile([128, 32], F32, tag="rstd")
        nmur = persist.tile([128, 32], F32, tag="nmur")

        xpT8 = xpT.bitcast(FP8)  # [128, 2, 2*NT]: (p, pc, (t,g))
        outT8 = outT.bitcast(FP8)  # [128, 2*NT]: (p, (t,g))
        kTflat = [kT[i].rearrange("p c t -> p (c t)") for i in range(2)]

        # ---------------- Phase A1: LN -> xhat fp8 -> transpose + pooling ----
        with tc.tile_pool(name="xpool", bufs=32) as pX, tc.tile_pool(
            name="xhpool", bufs=12
        ) as pXH, tc.tile_pool(name="stpool", bufs=4) as pST, tc.tile_pool(
            name="ps_pool", bufs=2, space="PSUM"
        ) as psum_pool:
            xts = []
            for t in range(32):
                xt = pX.tile([128, D], BF16, tag="xt", name=f"xt{t}")
                nc.sync.dma_start(
                    out=xt, in_=x.rearrange("(c p) d -> p c d", p=128)[:, t, :]
                )
                xts.append(xt)
            # heavy consts + init DMAs after x loads (x is the critical path)
            nc.sync.dma_start(out=wqk_sb, in_=wqk)
            nc.sync.dma_start(out=wv_sb, in_=wv)
            nc.sync.dma_start(out=bands_sb, in_=bandsd)
            nc.sync.dma_start(out=aI_sb, in_=alphaI.rearrange("a c p j -> p a c j"))
            nc.sync.dma_start(out=wout_sb, in_=wout)
            for i in range(2):
                nc.sync.dma_start(out=qT[i][:, 1, :], in_=zeros8)
                nc.sync.dma_start(out=kT[i][:, 1, :], in_=zeros8)
                nc.gpsimd.memset(qlT[i][:, 1, :], 0.0)
                nc.gpsimd.memset(klT[i][:, 1, :], 0.0)
            for t in range(32):
                stats = pST.tile([128, 6], F32, tag="stats", name=f"st{t}")
                nc.vector.bn_stats(out=stats, in_=xts[t])
                nc.vector.bn_aggr(out=mvall[:, :, t], in_=stats)
            vpe = pST.tile([128, 32], F32, tag="vpe")
            nc.vector.tensor_scalar_add(vpe, mvall[:, 1, :], EPS)
            lnv = pST.tile([128, 32], F32, tag="lnv")
            nc.scalar.activation(lnv, vpe, AF.Ln)
            nc.scalar.activation(rstd, lnv, AF.Exp, scale=-0.5)
            nc.vector.scalar_tensor_tensor(
                nmur, mvall[:, 0, :], -1.0, rstd, OP.mult, OP.mult,
            )
            for blk in range(4):
                if blk == 2:
                    nc.gpsimd.memset(v_nat[:, 0, :, :], 0.0)
                    nc.gpsimd.memset(v_nat[:, 33:35, :, :], 0.0)
                    nc.gpsimd.memset(v_nat[:, 1:33, :, 64:96], 0.0)
                    nc.gpsimd.memset(v_nat[:, 1:33, :, 64:65], 1.0)
                b8 = slice(blk * 8, blk * 8 + 8)
                psL = psum_pool.tile([1, 64], F32, tag="psL", name=f"psL{blk}")
                psL_sb = pST.tile([1, 64], BF16, tag="psLsb", name=f"psLsb{blk}")
                for tt in range(8):
                    t = blk * 8 + tt
                    nmurb = pST.tile([128, 1], BF16, tag="nmurb", name=f"nmurb{t}")
                    nc.gpsimd.tensor_copy(nmurb, nmur[:, t:t+1])
                    nc.tensor.matmul(
                        psL[:, tt * 8:(tt + 1) * 8], nmurb, poolm_sb,
                        start=True, stop=True,
                    )
                nc.vector.tensor_copy(psL_sb, psL)
                for tt in range(8):
                    t = blk * 8 + tt
                    pmr = pST.tile([128, 8], BF16, tag="pmr", name=f"pmr{t}")
                    nc.gpsimd.tensor_scalar_mul(pmr, poolm_sb, rstd[:, t:t+1])
                    psp = psum_pool.tile([128, 2, 2, 8], F32, tag="psp", name=f"psp{t}")
                    for pc in range(2):
                        for g in range(2):
                            sel = xts[t][:, pc * 256 : pc * 256 + 256].rearrange(
                                "p (m two) -> p m two", two=2
                            )[:, :, g]
                            nc.tensor.matmul(
                                psp[:, pc, g, :], sel, pmr,
                                start=True, stop=False,
                            )
                            nc.tensor.matmul(
                                psp[:, pc, g, :], ones_row,
                                psL_sb[:, tt * 8:(tt + 1) * 8],
                                start=False, stop=True,
                            )
                    nc.vector.tensor_copy(xlp[:, :, :, t * 8:(t + 1) * 8], psp)
                # LN applies + transposes for this blk (3-way engine split)
                for tt in range(8):
                    t = blk * 8 + tt
                    xh = pXH.tile([128, D], FP8, tag="xh", name=f"xh{t}")
                    if t % 8 < 3:
                        nc.scalar.activation(
                            xh, xts[t], AF.Identity,
                            bias=nmur[:, t:t+1], scale=rstd[:, t:t+1],
                        )
                    else:
                        nc.gpsimd.tensor_scalar(
                            xh, xts[t], mvall[:, 0, t:t+1], rstd[:, t:t+1],
                            OP.subtract, OP.mult,
                        )
                    nc.sync.dma_start_transpose(
                        out=xpT[:, :, t * 128:(t + 1) * 128], in_=xh.bitcast(U16)
                    )

        # ---------------- phase functions ------------------------------------
        HS = {}

        def lm_proj(pMid):
            # landmark q/k projections: psq rows = both heads of the pair
            for cc in range(4):
                dst = (qlT if cc < 2 else klT)[cc % 2]
                psq = pMid.tile([128, M], F32, tag="mid", name=f"psq{cc}")
                for pc in range(2):
                    nc.tensor.matmul(
                        psq,
                        wqk_sb[:, pc, :, (cc // 2) * 256 + (cc % 2) * 128 : (cc // 2) * 256 + (cc % 2) * 128 + 128],
                        xlp[:, pc, :, :],
                        start=(pc == 0), stop=(pc == 1), perf_mode=DR,
                    )
                nc.vector.tensor_copy(dst[:, 0, :], psq)

        def qkv_gen(pA2):
            rr = [0]

            def evac(out, in_, scale=None):
                rr[0] += 1
                if rr[0] % 2 == 0:
                    if scale is None:
                        nc.vector.tensor_copy(out, in_)
                    else:
                        nc.vector.tensor_scalar_mul(out, in_, scale)
                else:
                    nc.scalar.activation(
                        out, in_, AF.Copy, scale=1.0 if scale is None else scale
                    )

            for t8 in range(8):
                sp = t8 * 512
                for cc in range(4):
                    ps = pA2.tile([128, 512], F32, tag="a2", name=f"psqk{t8}_{cc}")
                    rhs = xpT8[:, :, 2 * sp : 2 * sp + 1024].rearrange(
                        "p c (t g) -> p c g t", g=2
                    )
                    for pc in range(2):
                        nc.tensor.matmul(
                            ps,
                            wqk_sb[:, pc, :, (cc // 2) * 256 + (cc % 2) * 128 : (cc // 2) * 256 + (cc % 2) * 128 + 128],
                            rhs[:, pc, :, :],
                            start=(pc == 0), stop=(pc == 1), perf_mode=DR,
                        )
                    dst = (qT if cc < 2 else kT)[cc % 2]
                    evac(dst[:, 0, sp:sp + 512], ps)
                for tv2 in range(2):
                    psv = pA2.tile([128, 2, 256], F32, tag="a2", name=f"psv{t8}_{tv2}")
                    for ti in range(2):
                        tv = t8 * 4 + tv2 * 2 + ti
                        for g in range(2):
                            lhsT = xpT8[
                                :, :, 2 * tv * 128 : 2 * tv * 128 + 256
                            ].rearrange("p c (t g) -> p c g t", g=2)[:, :, g, :]
                            nc.tensor.matmul(
                                psv[:, ti, :], lhsT, wv_sb[:, g, :, :],
                                start=(g == 0), stop=(g == 1), perf_mode=DR,
                            )
                    tv0 = t8 * 4 + tv2 * 2
                    evac(
                        v_nat[:, tv0 + 1: tv0 + 3, :, 0:64],
                        psv.rearrange("p c (h d) -> p c h d", h=HC),
                        scale=1.0 / SV,
                    )
                yield

        def ph_attn2(h, pMid, pS):
            st = HS[h]
            ps2 = pMid.tile([128, 2, 256], F32, tag="mid", name=f"ps2_{h}")
            for ic in range(2):
                nc.tensor.matmul(
                    ps2[:, ic, :],
                    st["qlTh"][:, :, ic * 128:(ic + 1) * 128],
                    st["klTh"],
                    start=True, stop=True, perf_mode=DR,
                )
            e2 = pS.tile([128, 2, 256], BF16, tag="e2", name=f"e2_{h}")
            nc.scalar.activation(e2, ps2, AF.Exp, scale=ES)
            rs = pS.tile([128, 2], F32, tag="rs", name=f"rs_{h}")
            nc.vector.tensor_reduce(rs, e2, mybir.AxisListType.X, OP.add)
            rr2 = pS.tile([128, 2], F32, tag="rr2", name=f"rr2_{h}")
            nc.vector.reciprocal(rr2, rs)
            a_nat = [
                st["pa"].tile([128, M], BF16, tag=f"anat{ic}", name=f"anat{h}_{ic}")
                for ic in range(2)
            ]
            rr2b = pS.tile([128, 2], F32, tag="rr2b", name=f"rr2b_{h}")
            nc.vector.tensor_copy(rr2b, rr2)
            for ic in range(2):
                nc.gpsimd.tensor_scalar_mul(a_nat[ic], e2[:, ic, :], rr2b[:, ic:ic+1])
            st["a_nat"] = a_nat

        def ph_z0(h, pMid, pS):
            st = HS[h]
            a_nat = st["a_nat"]
            pZ, pa = st["pZ"], st["pa"]
            psc = pMid.tile([1, M], F32, tag="mid", name=f"psc_{h}")
            for ic in range(2):
                nc.tensor.matmul(
                    psc, ones128, a_nat[ic], start=(ic == 0), stop=(ic == 1)
                )
            cmax = pS.tile([1, 1], F32, tag="cmax", name=f"cmax_{h}")
            nc.vector.tensor_reduce(cmax, psc, mybir.AxisListType.X, OP.max)
            crec = pS.tile([1, 1], BF16, tag="crec", name=f"crec_{h}")
            nc.vector.reciprocal(crec, cmax)
            crec_b = pS.tile([128, 1], F32, tag="crecb", name=f"crecb_{h}")
            psb_ = pMid.tile([128, 128], F32, tag="mid", name=f"psb_{h}")
            nc.tensor.matmul(psb_[:, 0:1], neg_row, crec, start=True, stop=True)
            nc.vector.tensor_copy(crec_b, psb_[:, 0:1])
            aT = pa.tile([128, 2, M], BF16, tag="aT", name=f"aT_{h}")
            z = pZ.tile([128, 2, M], BF16, tag="z", name=f"z_{h}")
            zT = pZ.tile([128, 2, M], BF16, tag="zT", name=f"zT_{h}")
            for jc in range(2):
                pT = pMid.tile([128, 2, 128], BF16, tag="mid", name=f"pTa_{h}_{jc}")
                for ic in range(2):
                    nc.tensor.transpose(
                        pT[:, ic, :], a_nat[ic][:, jc * 128:(jc + 1) * 128], ident_sb
                    )
                nc.vector.tensor_copy(aT[:, jc, :], pT.rearrange("p a b -> p (a b)"))
                nc.gpsimd.tensor_scalar_mul(z[:, jc, :], aT[:, jc, :], crec_b)
            for ic in range(2):
                nc.gpsimd.tensor_scalar_mul(zT[:, ic, :], a_nat[ic], crec_b)
            st["aT"], st["z"], st["zT"] = aT, z, zT

        def e3_scores(h, quad, pBig, pE3):
            st = HS[h]
            hp, ki = st["hp"], st["pair"]
            ps3 = pBig.tile([128, 4, 256], F32, tag="big", name=f"ps3_{h}_{quad}")
            for i in range(4):
                c = quad * 4 + i
                lhsT = kTflat[ki][hp:hp + 64, c * 128 : c * 128 + 256].rearrange(
                    "p (c2 t) -> p c2 t", c2=2
                )
                nc.tensor.matmul(
                    ps3[:, i, :], lhsT, st["qlTh"],
                    start=True, stop=True, perf_mode=DR,
                )
            e3q = pE3.tile([128, 4, 256], FP8, tag="e3q", name=f"e3q_{h}_{quad}")
            nc.scalar.activation(e3q, ps3, AF.Exp, scale=ES)
            st["e3qs"][quad] = e3q

        def e3_o2(h, quad):
            st = HS[h]
            e3q = st["e3qs"].pop(quad)
            o2ps = st["o2ps"]
            for i2 in range(2):
                pr = quad * 2 + i2
                nc.tensor.matmul(
                    o2ps,
                    v_nat[:, 2 * pr + 1 : 2 * pr + 3, h, :],
                    e3q[:, 2 * i2 : 2 * i2 + 2, :],
                    start=(pr == 0), stop=(pr == 15), perf_mode=DR,
                )

        def ph_e3_fin(h, pMid, pS):
            st = HS[h]
            o2ps = st["o2ps"]
            rrow = pS.tile([1, M], BF16, tag="rrow", name=f"rrow_{h}")
            nc.vector.reciprocal(rrow, o2ps[64:65, :])
            rs3 = [
                pS.tile([128, 1], F32, tag=f"rs3{jc}", name=f"rs3_{h}_{jc}")
                for jc in range(2)
            ]
            for jc in range(2):
                pT = pMid.tile([128, 128], BF16, tag="mid", name=f"pTf_{h}_{jc}")
                nc.tensor.transpose(
                    pT[:, 0:1], rrow[:, jc * 128:(jc + 1) * 128], ident_sb[0:1, 0:1]
                )
                nc.vector.tensor_scalar_mul(rs3[jc], pT[:, 0:1], -0.25)
            o2sb = pS.tile([64, M], BF16, tag="o2sb", name=f"o2sb_{h}")
            nc.vector.tensor_copy(o2sb, o2ps[0:64, :])
            o2n = [
                pS.tile([128, DH], BF16, tag=f"o2n{jc}", name=f"o2n_{h}_{jc}")
                for jc in range(2)
            ]
            for jc in range(2):
                pT = pMid.tile([128, 128], BF16, tag="mid", name=f"pTg_{h}_{jc}")
                nc.tensor.transpose(
                    pT[:, 0:64], o2sb[:, jc * 128:(jc + 1) * 128],
                    ident_sb[0:64, 0:64],
                )
                nc.vector.tensor_scalar_mul(o2n[jc], pT[:, 0:64], rs3[jc])
            st["o2n"] = o2n

        def ph_pinv_iter(h, pMid):
            # z/zT carry a flipped sign (-1/c); psums are seeded with alpha*I
            # so u_i = plain psum copies (Act-eligible for pair 1).
            st = HS[h]
            aT, z, zT = st["aT"], st["z"], st["zT"]
            pU = st["pU"]

            def cp(out, in_):
                if st["pair"] == 0:
                    nc.vector.tensor_copy(out, in_)
                else:
                    nc.scalar.activation(out, in_, AF.Copy)

            azT = pU.tile([128, 2, M], BF16, tag="u", name=f"azT_{h}")
            u1 = pU.tile([128, 2, M], BF16, tag="u", name=f"u1_{h}")
            ps_az = pMid.tile([128, 2, M], F32, tag="mid", name=f"ps_az_{h}")
            for oc in range(2):
                nc.tensor.matmul(
                    ps_az[:, oc, :], ident_sb, aI_sb[:, 0, oc, :],
                    start=True, stop=False,
                )
                for kc in range(2):
                    nc.tensor.matmul(
                        ps_az[:, oc, :],
                        aT[:, kc, oc * 128:(oc + 1) * 128],
                        z[:, kc, :],
                        start=False, stop=(kc == 1),
                    )
            cp(u1.rearrange("p a b -> p (a b)"), ps_az.rearrange("p a b -> p (a b)"))
            ps_azT = pMid.tile([128, 2, M], F32, tag="mid", name=f"ps_azT_{h}")
            for oc in range(2):
                for kc in range(2):
                    nc.tensor.matmul(
                        ps_azT[:, oc, :],
                        z[:, kc, oc * 128:(oc + 1) * 128],
                        aT[:, kc, :],
                        start=(kc == 0), stop=(kc == 1),
                    )
            cp(azT.rearrange("p a b -> p (a b)"), ps_azT.rearrange("p a b -> p (a b)"))
            u2 = pU.tile([128, 2, M], BF16, tag="u", name=f"u2_{h}")
            ps_p1 = pMid.tile([128, 2, M], F32, tag="mid", name=f"ps_p1_{h}")
            for oc in range(2):
                nc.tensor.matmul(
                    ps_p1[:, oc, :], ident_sb, aI_sb[:, 1, oc, :],
                    start=True, stop=False,
                )
                for kc in range(2):
                    nc.tensor.matmul(
                        ps_p1[:, oc, :],
                        azT[:, kc, oc * 128:(oc + 1) * 128],
                        u1[:, kc, :],
                        start=False, stop=(kc == 1),
                    )
            cp(u2.rearrange("p a b -> p (a b)"), ps_p1.rearrange("p a b -> p (a b)"))
            u3 = pU.tile([128, 2, M], BF16, tag="u", name=f"u3_{h}")
            ps_p2 = pMid.tile([128, 2, M], F32, tag="mid", name=f"ps_p2_{h}")
            for oc in range(2):
                nc.tensor.matmul(
                    ps_p2[:, oc, :], ident_sb, aI_sb[:, 2, oc, :],
                    start=True, stop=False,
                )
                for kc in range(2):
                    nc.tensor.matmul(
                        ps_p2[:, oc, :],
                        azT[:, kc, oc * 128:(oc + 1) * 128],
                        u2[:, kc, :],
                        start=False, stop=(kc == 1),
                    )
            cp(u3.rearrange("p a b -> p (a b)"), ps_p2.rearrange("p a b -> p (a b)"))
            zTn = st["pZ"].tile([128, 2, M], BF16, tag="zT", name=f"zTn_{h}")
            ps_zTn = pMid.tile([128, 2, M], F32, tag="mid", name=f"ps_zTn_{h}")
            for oc in range(2):
                for kc in range(2):
                    nc.tensor.matmul(
                        ps_zTn[:, oc, :],
                        u3[:, kc, oc * 128:(oc + 1) * 128],
                        zT[:, kc, :],
                        start=(kc == 0), stop=(kc == 1),
                    )
            cp(zTn.rearrange("p a b -> p (a b)"), ps_zTn.rearrange("p a b -> p (a b)"))
            st["zT"] = zTn

        def ph_C(h, pMid):
            st = HS[h]
            zT, o2n = st["zT"], st["o2n"]
            Cp = st["pa"].tile([128, 2, 96], FP8, tag="Cp", name=f"Cp_{h}")
            nc.gpsimd.memset(Cp[:, :, 64:96], 0.0)
            nc.gpsimd.memset(Cp[:, :, 64:65], 1.0)
            for ic in range(2):
                ps = pMid.tile([128, 64], F32, tag="mid", name=f"psC_{h}_{ic}")
                for jc in range(2):
                    nc.tensor.matmul(
                        ps,
                        zT[:, jc, ic * 128:(ic + 1) * 128],
                        o2n[jc],
                        start=(jc == 0), stop=(jc == 1),
                    )
                nc.vector.tensor_copy(Cp[:, ic, 0:64], ps)
            st["Cp"] = Cp

        def e1_t8(h, t8, pBig, pE1):
            st = HS[h]
            hp, ki = st["hp"], st["pair"]
            sp = t8 * 512
            psE = pBig.tile([128, 2, 512], F32, tag="big", name=f"psE_{h}_{t8}")
            for jc in range(2):
                nc.tensor.matmul(
                    psE[:, jc, :],
                    st["klTh"][:, :, jc * 128:(jc + 1) * 128],
                    qT[ki][hp:hp + 64, :, sp:sp + 512],
                    start=True, stop=True, perf_mode=DR,
                )
            e1q = pE1.tile([128, 2, 512], FP8, tag="e1q", name=f"e1q_{h}_{t8}")
            nc.scalar.activation(e1q, psE, AF.Exp, scale=ES)
            st["e1qs"][t8] = e1q

        def out_chunk(pair, heads, ck, pA2, pS):
            t8, tq = ck // 4, ck % 4
            psO = pA2.tile([128, 2, 160], F32, tag="a2", name=f"psO_{pair}_{ck}")
            for hi, h in enumerate(heads):
                st = HS[h]
                e1q = st["e1qs"][t8]
                nc.tensor.matmul(
                    psO[:, hi, 0:96],
                    e1q[:, :, tq * 128:(tq + 1) * 128],
                    st["Cp"],
                    start=True, stop=True, perf_mode=DR,
                )
                for p2 in range(2):
                    nc.tensor.matmul(
                        psO[:, hi, 96:160],
                        bands_sb[:, h, 2 * p2 : 2 * p2 + 2, :],
                        v_nat[:, ck + 2 * p2 : ck + 2 * p2 + 2, h, 0:64],
                        start=(p2 == 0), stop=(p2 == 1), perf_mode=DR,
                    )
            cnv = pS.tile([128, 2, DH], BF16, tag="cnv", name=f"cnv_{pair}_{ck}")
            nc.scalar.activation(cnv, psO[:, :, 96:160], AF.Copy)
            rr = pS.tile([128, 2], F32, tag="rrO", name=f"rrO_{pair}_{ck}")
            nc.vector.reciprocal(rr, psO[:, :, 64])
            for hi, h in enumerate(heads):
                nc.vector.scalar_tensor_tensor(
                    out_nat[:, ck, h, :],
                    psO[:, hi, 0:64],
                    rr[:, hi:hi + 1],
                    cnv[:, hi, :],
                    OP.mult,
                    OP.add,
                )

        def e1_pair(pair, heads, pBig, pE1, pA2, pS):
            for h in heads:
                HS[h]["e1qs"] = {}
            for h in heads:
                e1_t8(h, 0, pBig, pE1)
            for t8 in range(8):
                if t8 < 7:
                    for h in heads:
                        e1_t8(h, t8 + 1, pBig, pE1)
                for tq in range(4):
                    out_chunk(pair, heads, t8 * 4 + tq, pA2, pS)
                yield

        def fin_chain(heads, pMid, pS):
            for h in heads:
                ph_e3_fin(h, pMid, pS)
                yield
            for h in heads:
                ph_pinv_iter(h, pMid)
                yield
            for h in heads:
                ph_C(h, pMid)
                yield

        def e3_pair(heads, pBig, pE3, pO2):
            o2pair = pO2.tile([96, 2, 256], F32, tag="o2", name=f"o2ps_{heads[0]}")
            for h in heads:
                HS[h]["o2ps"] = o2pair[:, h % 2, :]
                HS[h]["e3qs"] = {}
            for h in heads:
                e3_scores(h, 0, pBig, pE3)
            for quad in range(8):
                for h in heads:
                    if quad < 7:
                        e3_scores(h, quad + 1, pBig, pE3)
                    e3_o2(h, quad)
                yield

        def drain_gen(pMid, pO2, pFO):
            opr = out_p.rearrange("(c p) d -> p c d", p=128)
            for t8 in range(8):
                ck0 = t8 * 4
                nc.sync.dma_start_transpose(
                    out=outT[:, ck0 * 128:(ck0 + 4) * 128].rearrange(
                        "p (c t) -> p c t", c=4
                    ),
                    in_=out_nat[:, ck0:ck0 + 4, :, :].rearrange(
                        "p c h d -> p (c h d)"
                    ).bitcast(U16),
                )
                for ck2 in range(2 * t8, 2 * t8 + 2):
                    fo = pFO.tile([128, 2, 512], BF16, tag="fo", name=f"fo{ck2}")
                    for ci in range(2):
                        ck = 2 * ck2 + ci
                        pool_ = pMid if ck % 2 == 0 else pO2
                        tag_ = "mid" if ck % 2 == 0 else "o2"
                        psF = pool_.tile([128, 512], F32, tag=tag_, name=f"psF_{ck}")
                        lhsT2 = outT8[:, 2 * ck * 128 : 2 * ck * 128 + 256].rearrange(
                            "p (t g) -> p g t", g=2
                        )
                        for g in range(2):
                            nc.tensor.matmul(
                                psF, lhsT2[:, g, :], wout_sb[:, g, :],
                                start=(g == 0), stop=(g == 1),
                            )
                        if ck % 2 == 0:
                            nc.vector.tensor_scalar_mul(fo[:, ci, :], psF, 1.0 / SO)
                        else:
                            nc.scalar.activation(
                                fo[:, ci, :], psF, AF.Copy, scale=1.0 / SO
                            )
                    nc.sync.dma_start(out=opr[:, 2 * ck2 : 2 * ck2 + 2, :], in_=fo)
                yield

        # ---------------- heads phase orchestration --------------------------
        with tc.tile_pool(name="ps_big", bufs=2, space="PSUM") as pBig, tc.tile_pool(
            name="ps_a2", bufs=2, space="PSUM"
        ) as pA2, tc.tile_pool(
            name="ps_o2", bufs=1, space="PSUM"
        ) as pO2, tc.tile_pool(
            name="ps_mid", bufs=1, space="PSUM"
        ) as pMid, tc.tile_pool(name="headS", bufs=8) as pS, tc.tile_pool(
            name="head_a", bufs=6
        ) as pa_pool, tc.tile_pool(name="pinv_u", bufs=6) as pU, tc.tile_pool(
            name="pinv_z", bufs=6
        ) as pZ, tc.tile_pool(name="e1pool", bufs=6) as pE1, tc.tile_pool(
            name="e3pool", bufs=4
        ) as pE3, tc.tile_pool(name="fo", bufs=4) as pFO:
            for h in range(4):
                pair, hp = h // 2, 64 * (h % 2)
                HS[h] = {
                    "pair": pair, "hp": hp,
                    "qlTh": qlT[pair][hp:hp + 64, :, :],
                    "klTh": klT[pair][hp:hp + 64, :, :],
                    "pZ": pZ, "pU": pU, "pa": pa_pool,
                }

            lm_proj(pMid)
            gqkv = qkv_gen(pA2)
            next(gqkv, None)
            next(gqkv, None)
            next(gqkv, None)
            for h in range(4):
                ph_attn2(h, pMid, pS)
            for h in range(4):
                ph_z0(h, pMid, pS)
            # e3 pair0 || qkv
            g_e3a = e3_pair([0, 1], pBig, pE3, pO2)
            for quad in range(8):
                next(g_e3a, None)
                next(gqkv, None)
            # e3 pair1 || fin pair0 || qkv tail
            g_e3b = e3_pair([2, 3], pBig, pE3, pO2)
            g_fin0 = fin_chain([0, 1], pMid, pS)
            for quad in range(8):
                next(g_e3b, None)
                next(g_fin0, None)
                next(gqkv, None)
            for _ in g_fin0:
                pass
            for _ in gqkv:
                pass
            # e1 pair0 || fin pair1
            g_e1a = e1_pair(0, [0, 1], pBig, pE1, pA2, pS)
            g_fin1 = fin_chain([2, 3], pMid, pS)
            for t8 in range(8):
                next(g_e1a, None)
                next(g_fin1, None)
            for _ in g_fin1:
                pass
            # e1 pair1 || drain
            g_e1b = e1_pair(1, [2, 3], pBig, pE1, pA2, pS)
            g_dr = drain_gen(pMid, pO2, pFO)
            for t8 in range(8):
                next(g_e1b, None)
                next(g_dr, None)
            for _ in g_dr:
                pass
    lp.__exit__(None, None, None)


_NC_CACHE = None


def build_nc():
    global _NC_CACHE
    if _NC_CACHE is not None:
        return _NC_CACHE
    nc = bacc.Bacc("TRN2", target_bir_lowering=False, debug=False, num_devices=8)
    with tile.TileContext(nc) as tc:
        build_kernel_body(tc)
    nc.compile()
    _NC_CACHE = nc
    return nc


def host_inputs(x, w_qkv, w_out, b_out, res_w, ln_g, ln_b):
    """Build the 8 per-core input maps."""
    assert np.abs(ln_b).max() == 0.0, "nonzero ln_b not supported"
    import ml_dtypes

    bf16 = ml_dtypes.bfloat16
    fp8 = ml_dtypes.float8_e4m3
    eye = np.eye(M, dtype=np.float32)
    alphaI = np.stack(
        [a * eye.reshape(2, 128, M) for a in (7.0, 15.0, 13.0)]
    ).astype(bf16)
    identity = np.eye(128, dtype=bf16)
    poolm = np.zeros((128, 8), dtype=np.float32)
    for t in range(128):
        poolm[t, t // L] = 1.0 / L
    poolm = poolm.astype(bf16)
    zeros8 = np.zeros((128, NT), dtype=fp8)

    p_idx = np.arange(128)
    tp = np.arange(128)[:, None]
    t_ = np.arange(128)[None, :]
    in_maps = []
    for c in range(8):
        b, g = c // 2, c % 2
        qsl = slice(g * 256, g * 256 + 256)
        ksl = slice(512 + g * 256, 512 + g * 256 + 256)
        vsl = slice(1024 + g * 256, 1024 + g * 256 + 256)
        wq = (ln_g[:, None] * w_qkv[:, qsl]) * SQ  # [512, 256]
        wk = (ln_g[:, None] * w_qkv[:, ksl]) * SK
        wv_ = (ln_g[:, None] * w_qkv[:, vsl]) * SV
        # d-pair layout: [p, pc, gg, col], d = 256*pc + 2*p + gg
        wqk_c = np.zeros((128, 2, 2, 512), dtype=np.float32)
        wv_c = np.zeros((128, 2, 2, 256), dtype=np.float32)
        for pc in range(2):
            for gg in range(2):
                rows = 256 * pc + 2 * p_idx + gg
                wqk_c[:, pc, gg, 0:256] = wq[rows, :]
                wqk_c[:, pc, gg, 256:512] = wk[rows, :]
                wv_c[:, gg, pc, :] = wv_[rows, :]
        # wout: hd-pair layout [p, gg, :], hd = 2*p + gg (within group g)
        wout_c = np.zeros((128, 2, 512), dtype=np.float32)
        for gg in range(2):
            wout_c[:, gg, :] = w_out[g * 256 + 2 * p_idx + gg, :] * SO
        bands = np.zeros((128, HC, 4, 128), dtype=np.float32)
        for i in range(HC):
            w33 = res_w[4 * g + i, 0, :, 0]
            for pos, off in ((0, -128), (1, 0), (2, 128)):
                k = (tp + off) - t_ + 16
                msk = (k >= 0) & (k < KW)
                bands[:, i, pos][msk] = w33[k[msk]]
        in_maps.append(
            {
                "x": np.ascontiguousarray(x[b], dtype=bf16),
                "wqk": wqk_c.astype(fp8),
                "wv": wv_c.astype(fp8),
                "wout": wout_c.astype(fp8),
                "bands": bands.astype(fp8),
                "alphaI": alphaI,
                "ident": identity,
                "poolm": poolm,
                "zeros8": zeros8,
            }
        )
    return in_maps


def run(inputs, trace=False):
    nc = build_nc()
    in_maps = host_inputs(**inputs)
    res = run_bass_kernel_spmd(nc, in_maps, list(range(8)), trace=trace)
    x = inputs["x"]
    b_out = inputs["b_out"]
    out = np.stack(
        [
            res.results[2 * b]["out_partial"].astype(np.float32)
            + res.results[2 * b + 1]["out_partial"].astype(np.float32)
            for b in range(4)
        ]
    )
    out = out + x + b_out[None, None, :]
    return out.astype(np.float32), res


def kernel(**inputs):
    out, _ = run(inputs, trace=False)
    return out
